# revision 1
# baseline (speedup 1.0000x reference)
"""BiLSTM seq2seq + Bahdanau attention + vocab softmax on 8 trn2 NeuronCores.

Strategy (one uniform SPMD program; all divergence lives in per-core input data):
  - encoder fwd LSTM on cores 0-3, bwd on cores 4-7 (bwd cores receive
    time-reversed token indices; downstream attention is order-blind in s,
    so the scan-order storage never needs re-reversal)
  - pairwise AllGather exchanges the two encoder halves
  - decoder LSTM replicated on all cores (per-step cost is weight-streaming
    bound into the PE and independent of batch, so replication is free
    parallelism; collectives have a ~20us latency floor so per-step
    tensor-parallel sync is impossible)
  - attention token-sharded 8 ways; softmax-normalization of attention is
    deferred and folded into the output-dense scaling (per-partition scalar)
  - output dense vocab-sharded 8 ways in bf16; vocab softmax via one
    AllReduce of per-token partial sums
Recurrence matmuls run "orientation A": z^T[gates, B] tiles with the weight
tile stationary (bf16 -> fast weight load) and h^T streaming, which leaves z
in [gate-dim-on-partitions, batch] layout so all gate nonlinearities are
full-width engine ops.
"""

import os
import numpy as np
import ml_dtypes
from contextlib import ExitStack

import concourse.bass as bass
import concourse.tile as tile
from concourse import mybir
from concourse.bass_utils import run_bass_kernel_spmd
from concourse.masks import make_identity

FP32 = mybir.dt.float32
BF16 = mybir.dt.bfloat16
I32 = mybir.dt.int32
AF = mybir.ActivationFunctionType
ALU = mybir.AluOpType
ENG = mybir.EngineType

NC = 8
B = 4
TIN = 128
TOUT = 128
E = 512
H = 512
D = 2 * H            # 1024
V = 32000
VSH = V // NC        # 4000
TPC = TOUT // NC     # 16 token-positions per core
NTOK = B * TOUT      # 512 (b, t) pairs
TOKC = NTOK // NC    # 64 tokens per core
EM = E // 128        # 4 chunks of the embedding dim
KM = H // 128        # 4 K-chunks (encoder recurrence)
KD = D // 128        # 8 K-chunks (decoder recurrence)
ME = 4 * H // 128    # 16 gate m-tiles (encoder)
MD = 4 * D // 128    # 32 gate m-tiles (decoder)
NV = 8               # vocab free-chunks per core
VW = VSH // NV       # 500
AGR = D + 8          # allgather rows: 1024 attn + row 1024 = denom + pad


def sq(ap):
    """Merge trailing count-1 free dims (shape-match helper)."""
    n = len(ap.ap) - 1  # free dims
    names = " ".join(f"a{i}" for i in range(n))
    merged = f"a0 ({' '.join(f'a{i}' for i in range(1, n))})"
    return ap.rearrange(f"p {names} -> p {merged}")


def sqz(ap):
    """Repeatedly merge a trailing count-1 dim into its predecessor."""
    while len(ap.ap) > 2 and ap.ap[-1][1] == 1:
        n = len(ap.ap) - 1
        names = [f"a{i}" for i in range(n)]
        lhs = "p " + " ".join(names)
        rhs = "p " + " ".join(names[:-2] + [f"({names[-2]} {names[-1]})"])
        ap = ap.rearrange(f"{lhs} -> {rhs}")
    return ap


def legalize_waits(nc, max_waits=1):
    """This walrus build accepts at most `max_waits` sync-wait commands per
    instruction; hoist excess waits onto injected same-engine NoOps."""
    n = 0

    def make_nop(engine, wait):
        eng = nc.engines[engine]
        inst = eng.nop(nofuse=True).ins
        bb = nc.cur_bb.bb
        lst = bb.instructions
        assert lst and lst[-1].name == inst.name
        lst.pop()
        bb.instructions = lst
        inst.sync_info = mybir.SyncInfo(on_wait=[wait], on_update=[])
        return inst

    for blk in nc.main_func.blocks:
        new_insts = []
        changed = False
        for inst in blk.instructions:
            si = inst.sync_info
            waits = list(si.on_wait) if si and si.on_wait else []
            if len(waits) > max_waits:
                excess, keep = waits[:-max_waits], waits[-max_waits:]
                for w in excess:
                    new_insts.append(make_nop(inst.engine, w))
                    n += 1
                si.on_wait = keep
                changed = True
            new_insts.append(inst)
        if changed:
            blk.instructions = new_insts
    return n


def build_program(debug=False, enc_unroll=4, dec_unroll=2, enc_steps=TIN, dec_steps=TOUT, reps=1):
    nc = bass.Bass("TRN2", target_bir_lowering=False, debug=False,
                   num_devices=NC)

    def din(name, shape, dt=FP32):
        return nc.dram_tensor(name, shape, dt, kind="ExternalInput").ap()

    def dout(name, shape, dt=FP32):
        return nc.dram_tensor(name, shape, dt, kind="ExternalOutput").ap()

    enc_mini = din("enc_mini", [NTOK, E])
    enc_idx = din("enc_idx", [128, EM], I32)
    dec_mini = din("dec_mini", [NTOK, E])
    dec_idx = din("dec_idx", [128, EM], I32)
    wx_m = din("wx_m", [E, 4 * H], BF16)
    wh_m = din("wh_m", [H, 4 * H], BF16)
    b_m = din("b_m", [128, ME])
    wx_d = din("wx_d", [E, 4 * D], BF16)
    wh_d = din("wh_d", [D, 4 * D], BF16)
    b_d = din("b_d", [128, MD])
    v_sc = din("v_sc", [128, KD], BF16)
    wo_sh = din("wo_sh", [D, VSH], BF16)
    tok = din("tok", [TPC, 1], I32)

    o_probs = dout("o_probs", [NTOK, VSH])
    if debug:
        o_enc = dout("o_enc", [128, 2, KM, B, TIN])
        o_dec = dout("o_dec", [128, KD, B, TOUT])
        o_attn = dout("o_attn", [NC, AGR, TOKC])

    with tile.TileContext(nc) as tc:
        # whole-run pools
        const = tc.alloc_tile_pool(name="const", bufs=1)
        work = tc.alloc_tile_pool(name="work", bufs=3)
        dram = tc.alloc_tile_pool(name="dram", bufs=1, space="DRAM")

        ident = const.tile([128, 128], FP32)
        make_identity(nc, ident[:])
        ones_col = const.tile([128, 1], FP32)
        nc.vector.memset(ones_col[:], 1.0)
        bm_sb = const.tile([128, ME], FP32)
        nc.sync.dma_start(bm_sb[:], b_m[:])
        bd_sb = const.tile([128, MD], FP32)
        nc.sync.dma_start(bd_sb[:], b_d[:])
        v_sb = const.tile([128, KD], BF16)
        nc.sync.dma_start(v_sb[:], v_sc[:])
        tok_sb = const.tile([TPC, 1], I32)
        nc.sync.dma_start(tok_sb[:], tok[:])

        # encoder-lifetime + decoder-lifetime pools
        dec_w = tc.alloc_tile_pool(name="dec_w", bufs=1)
        enc_w = tc.alloc_tile_pool(name="enc_w", bufs=1)
        whm_sb = enc_w.tile([128, KM, 4 * H], BF16)
        nc.sync.dma_start(
            whm_sb[:], wh_m[:].rearrange("(k p) g -> p k g", p=128))
        whd_sb = dec_w.tile([128, KD, 4 * D], BF16)
        nc.sync.dma_start(
            whd_sb[:], wh_d[:].rearrange("(k p) g -> p k g", p=128))
        xw_m = enc_w.tile([128, ME, B, TIN], BF16)
        xw_d = dec_w.tile([128, MD, B, TOUT], BF16)

        # ---------------- phase 0: gathers + input projections -----------
        ph0 = tc.alloc_tile_pool(name="ph0", bufs=1)
        ph0w = tc.alloc_tile_pool(name="ph0w", bufs=3)
        ph0p = tc.alloc_tile_pool(name="ph0p", bufs=2, space="PSUM")

        def gather_transpose(mini, idx_dram, xt_tile, idx_name):
            idx_sb = ph0.tile([128, EM], I32, name=idx_name)
            nc.sync.dma_start(idx_sb[:], idx_dram[:])
            for j in range(EM):  # 128-row batches of (b, t) rows
                rows = ph0w.tile([128, E], FP32, tag="gatrows")
                nc.gpsimd.indirect_dma_start(
                    out=rows[:], out_offset=None,
                    in_=mini[:],
                    in_offset=bass.IndirectOffsetOnAxis(
                        ap=idx_sb[:, j:j + 1], axis=0))
                for ech in range(EM):
                    tp = ph0p.tile([128, 128], FP32, tag="tp0")
                    nc.tensor.transpose(
                        out=tp[:], in_=rows[:, ech * 128:(ech + 1) * 128],
                        identity=ident[:])
                    nc.vector.tensor_copy(
                        xt_tile[:, ech, j * 128:(j + 1) * 128], tp[:])

        def project(wx_dram, xt_tile, nm, b_sb, xw_tile):
            for m in range(nm):
                pj = ph0p.tile([128, NTOK], FP32, tag="pj")
                for kblk in range(EM):
                    wxt = ph0w.tile([128, 128], BF16, tag="wxt")
                    nc.sync.dma_start(
                        wxt[:],
                        wx_dram[kblk * 128:(kblk + 1) * 128,
                                m * 128:(m + 1) * 128])
                    nc.tensor.matmul(
                        pj[:], wxt[:], xt_tile[:, kblk, :],
                        start=(kblk == 0), stop=(kblk == EM - 1))
                nc.scalar.activation(
                    xw_tile[:, m, :, :].rearrange("p b t -> p (b t)"),
                    pj[:], AF.Identity, bias=b_sb[:, m:m + 1])

        enc_xT = ph0.tile([128, EM, NTOK], BF16)
        gather_transpose(enc_mini, enc_idx, enc_xT, "idx_e")
        dec_xT = ph0.tile([128, EM, NTOK], BF16)
        gather_transpose(dec_mini, dec_idx, dec_xT, "idx_d")
        project(wx_m, enc_xT, ME, bm_sb, xw_m)
        project(wx_d, dec_xT, MD, bd_sb, xw_d)

        ph0p.release()
        ph0w.release()
        ph0.release()

        # ---------------- phase 1: encoder recurrence ---------------------
        ench = tc.alloc_tile_pool(name="ench", bufs=1)
        recp = tc.alloc_tile_pool(name="recp", bufs=2, space="PSUM")
        enc_half = ench.tile([128, KM, B, TIN], FP32)
        h_enc = ench.tile([128, KM, B], BF16)
        c_enc = ench.tile([128, KM, B], FP32)
        nc.vector.memset(h_enc[:], 0.0)
        nc.vector.memset(c_enc[:], 0.0)

        def lstm_step(iv, km, nm, wh_sb, xw, h_st, c_st, out_slice_fn):
            ps = recp.tile([128, nm, B], FP32, tag="rec_ps")
            for m in range(nm):
                for k in range(km):
                    nc.tensor.matmul(
                        ps[:, m, :], wh_sb[:, k, m * 128:(m + 1) * 128],
                        h_st[:, k, :], start=(k == 0), stop=(k == km - 1))
            z = work.tile([128, nm, B], FP32, tag="rec_z")
            nc.vector.tensor_tensor(
                out=z[:], in0=ps[:], in1=sq(xw[:, :, :, bass.ds(iv, 1)]),
                op=ALU.add)
            g = km  # m-tiles per gate; host packs gates as (i, f, o, g)
            sio = work.tile([128, 3 * g, B], FP32, tag="rec_sio")
            tg = work.tile([128, g, B], FP32, tag="rec_tg")
            nc.scalar.activation(sio[:], z[:, 0:3 * g, :], AF.Sigmoid)
            nc.scalar.activation(tg[:], z[:, 3 * g:4 * g, :], AF.Tanh)
            nc.vector.tensor_tensor(out=tg[:], in0=sio[:, 0:g, :], in1=tg[:],
                                    op=ALU.mult)
            nc.vector.tensor_tensor(out=c_st[:], in0=c_st[:],
                                    in1=sio[:, g:2 * g, :], op=ALU.mult)
            nc.vector.tensor_tensor(out=c_st[:], in0=c_st[:], in1=tg[:],
                                    op=ALU.add)
            tc_t = work.tile([128, g, B], FP32, tag="rec_tc")
            nc.scalar.activation(tc_t[:], c_st[:], AF.Tanh)
            nc.vector.tensor_tensor(out=h_st[:], in0=sio[:, 2 * g:3 * g, :],
                                    in1=tc_t[:], op=ALU.mult)
            nc.vector.tensor_copy(out_slice_fn(iv), h_st[:])

        def enc_store(iv):
            return sq(enc_half[:, :, :, bass.ds(iv, 1)])  # write target

        def enc_loop():
            tc.For_i_unrolled_general(
                0, enc_steps, 1,
                lambda iv0, unroll: [
                    lstm_step(iv0 + i, KM, ME, whm_sb, xw_m, h_enc, c_enc,
                              enc_store) for i in range(unroll)],
                max_unroll=enc_unroll, hint_engines=(ENG.PE,))
        if reps == 1:
            enc_loop()
        else:
            with tc.For_i(0, reps, 1):
                enc_loop()

        # ---------------- phase 2: exchange encoder halves ----------------
        ag1_in = dram.tile([128, KM, B, TIN], FP32)
        ag1_out = dram.tile([2, 128, KM, B, TIN], FP32)
        nc.sync.dma_start(ag1_in[:], enc_half[:])
        nc.gpsimd.collective_compute(
            "AllGather", ALU.bypass,
            ins=[ag1_in.opt()], outs=[ag1_out.opt()],
            replica_groups=[[0, 4], [1, 5], [2, 6], [3, 7]])
        ench.release()
        enc_w.release()

        mid = tc.alloc_tile_pool(name="mid", bufs=1)
        # enc_dmaj: [128 d%128, grp, dm, b, s];   d = (grp*KM + dm)*128 + p
        enc_dmaj = mid.tile([128, 2, KM, B, TIN], FP32)
        nc.sync.dma_start(
            enc_dmaj[:],
            ag1_out[:].rearrange("g p k b t -> p g k b t"))
        if debug:
            nc.sync.dma_start(o_enc[:], enc_dmaj[:])
        # enc_smaj: [128 s, b, d]
        enc_smaj = mid.tile([128, B, D], FP32)
        for b in range(B):
            for dg in range(KD):
                tp = recp.tile([128, 128], FP32, tag="tp2")
                nc.tensor.transpose(
                    out=tp[:], in_=enc_dmaj[:, dg // KM, dg % KM, b, :],
                    identity=ident[:])
                nc.vector.tensor_copy(
                    enc_smaj[:, b, dg * 128:(dg + 1) * 128], tp[:])
        h_dec = mid.tile([128, KD, B], BF16)
        c_dec = mid.tile([128, KD, B], FP32)
        nc.vector.tensor_copy(h_dec[:, 0:KM, :],
                              enc_dmaj[:, 0, :, :, TIN - 1])
        nc.vector.tensor_copy(h_dec[:, KM:2 * KM, :],
                              enc_dmaj[:, 1, :, :, 0])
        nc.vector.memset(c_dec[:], 0.0)

        # ---------------- phase 3: decoder recurrence ---------------------
        dec_outT = mid.tile([128, KD, B, TOUT], FP32)

        def dec_store(iv):
            return sq(dec_outT[:, :, :, bass.ds(iv, 1)])

        def dec_loop():
            tc.For_i_unrolled_general(
                0, dec_steps, 1,
                lambda iv0, unroll: [
                    lstm_step(iv0 + i, KD, MD, whd_sb, xw_d, h_dec, c_dec,
                              dec_store) for i in range(unroll)],
                max_unroll=dec_unroll, hint_engines=(ENG.PE,))
        if reps == 1:
            dec_loop()
        else:
            with tc.For_i(0, reps, 1):
                dec_loop()
        if debug:
            nc.sync.dma_start(o_dec[:], dec_outT[:])
        recp.release()
        recp2 = tc.alloc_tile_pool(name="recp2", bufs=2, space="PSUM")

        # ---------------- phase 4: attention (token shard) ----------------
        # select this core's 16 query columns via DRAM round-trip +
        # indirect row gather + PE transpose (no dynamic APs needed)
        dec_dram = dram.tile([TOUT, KD, B, 128], FP32)
        for dg in range(KD):
            for b in range(B):
                nc.sync.dma_start(
                    dec_dram[:, dg, b, :].rearrange("t p -> p t"),
                    dec_outT[:, dg, b, :])
        q_rows = mid.tile([TPC, KD * B * 128], FP32)
        nc.gpsimd.indirect_dma_start(
            out=q_rows[:], out_offset=None,
            in_=dec_dram[:].rearrange("t k b p -> t (k b p)"),
            in_offset=bass.IndirectOffsetOnAxis(ap=tok_sb[:, :1], axis=0))
        q_sb = mid.tile([128, KD, B, TPC], FP32)
        for dg in range(KD):
            for b in range(B):
                tp = recp2.tile([128, TPC], FP32, tag="tpq")
                nc.tensor.transpose(
                    out=tp[:],
                    in_=q_rows[:, (dg * B + b) * 128:(dg * B + b + 1) * 128],
                    identity=ident[0:TPC, 0:TPC])
                nc.vector.tensor_copy(q_sb[:, dg, b, :], tp[:])

        attnU = mid.tile([128, KD, B, TPC], FP32)
        dn_sb = mid.tile([1, B, TPC], FP32)
        att = tc.alloc_tile_pool(name="att", bufs=3)
        attp = tc.alloc_tile_pool(name="attp", bufs=2, space="PSUM")
        for b in range(B):
            sc_ps = attp.tile([128, TPC], FP32, tag="sc")
            for tl in range(TPC):
                for dg in range(KD):
                    mt = att.tile([128, 128], BF16, tag="mt")
                    nc.scalar.activation(
                        mt[:], enc_dmaj[:, dg // KM, dg % KM, b, :],
                        AF.Tanh, bias=q_sb[:, dg, b, tl:tl + 1])
                    nc.tensor.matmul(
                        sc_ps[:, tl:tl + 1], mt[:], v_sb[:, dg:dg + 1],
                        start=(dg == 0), stop=(dg == KD - 1))
            ew = att.tile([128, TPC], FP32, tag="ew")
            nc.scalar.activation(ew[:], sc_ps[:], AF.Exp)
            dn_ps = attp.tile([1, TPC], FP32, tag="dn")
            nc.tensor.matmul(dn_ps[:], ones_col[:], ew[:],
                             start=True, stop=True)
            nc.vector.tensor_copy(dn_sb[:, b, :], dn_ps[:])
            au_ps = attp.tile([128, KD, TPC], FP32, tag="au")
            for dg in range(KD):
                nc.tensor.matmul(
                    au_ps[:, dg, :],
                    enc_smaj[:, b, dg * 128:(dg + 1) * 128],
                    ew[:], start=True, stop=True)
            nc.vector.tensor_copy(attnU[:, :, b, :], au_ps[:])
        attp.release()
        att.release()
        recp2.release()

        ag2_in = dram.tile([AGR, TOKC], FP32)
        ag2_out = dram.tile([NC, AGR, TOKC], FP32)
        for k in range(KD):
            nc.sync.dma_start(
                ag2_in[k * 128:(k + 1) * 128, :],
                attnU[:, k, :, :].rearrange("p b t -> p (b t)"))
        nc.sync.dma_start(
            ag2_in[D:D + 1, :], dn_sb[:].rearrange("o b t -> o (b t)"))
        nc.gpsimd.collective_compute(
            "AllGather", ALU.bypass,
            ins=[ag2_in.opt()], outs=[ag2_out.opt()],
            replica_groups=[list(range(NC))])
        if debug:
            nc.sync.dma_start(o_attn[:], ag2_out[:])
        mid.release()
        dec_w.release()

        # ---------------- phase 5: dense + vocab softmax ------------------
        ph5 = tc.alloc_tile_pool(name="ph5", bufs=1)
        ph5w = tc.alloc_tile_pool(name="ph5w", bufs=3)
        ph5p = tc.alloc_tile_pool(name="ph5p", bufs=4, space="PSUM")
        attn_bf = ph5.tile([128, KD, NTOK], BF16)
        for k in range(KD):
            tmpa = ph5w.tile([128, NC, TOKC], FP32, tag="tmpa")
            nc.sync.dma_start(
                tmpa[:],
                ag2_out[:, k * 128:(k + 1) * 128, :]
                .rearrange("c p t -> p c t"))
            nc.vector.tensor_copy(
                attn_bf[:, k, :].rearrange("p (c t) -> p c t", c=NC),
                tmpa[:])
        # attention-softmax denominators -> per-token reciprocal [128, 4]
        recd = ph5.tile([128, 4], FP32)
        for m in range(4):
            for half in range(2):
                c2 = 2 * m + half
                nc.sync.dma_start(
                    recd[half * 64:(half + 1) * 64, m:m + 1],
                    ag2_out[c2, D:D + 1, :].rearrange("o t -> t o"))
        nc.vector.reciprocal(recd[:], recd[:])

        esum = ph5.tile([128, 4], FP32)
        eprobs = ph5.tile([128, 4, VSH], FP32)
        for m in range(4):
            for n in range(NV):
                dps = ph5p.tile([128, VW], FP32, tag="dps")
                for k in range(KD):
                    wt = ph5w.tile([128, VW], BF16, tag="wo_t")
                    nc.sync.dma_start(
                        wt[:],
                        wo_sh[k * 128:(k + 1) * 128, n * VW:(n + 1) * VW])
                    nc.tensor.matmul(
                        dps[:], attn_bf[:, k, m * 128:(m + 1) * 128],
                        wt[:], start=(k == 0), stop=(k == KD - 1))
                part = ph5w.tile([128, 1], FP32, tag="part")
                lg = ph5w.tile([128, VW], FP32, tag="lg")
                nc.vector.tensor_scalar_mul(lg[:], dps[:], recd[:, m:m + 1])
                nc.scalar.activation(
                    eprobs[:, m, n * VW:(n + 1) * VW], lg[:], AF.Exp,
                    accum_out=part[:, :1])
                if n == 0:
                    nc.vector.tensor_copy(esum[:, m:m + 1], part[:])
                else:
                    nc.vector.tensor_tensor(
                        out=esum[:, m:m + 1], in0=esum[:, m:m + 1],
                        in1=part[:], op=ALU.add)

        ar_in = dram.tile([4, 128], FP32)
        ar_out = dram.tile([4, 128], FP32)
        nc.sync.dma_start(ar_in[:].rearrange("m p -> p m"), esum[:])
        nc.gpsimd.collective_compute(
            "AllReduce", ALU.add,
            ins=[ar_in.opt()], outs=[ar_out.opt()],
            replica_groups=[list(range(NC))])
        stot = ph5.tile([128, 4], FP32)
        nc.sync.dma_start(stot[:], ar_out[:].rearrange("m p -> p m"))
        nc.vector.reciprocal(stot[:], stot[:])
        for m in range(4):
            for n in range(NV):
                ob = ph5w.tile([128, VW], FP32, tag="ob")
                nc.vector.tensor_scalar_mul(
                    ob[:], eprobs[:, m, n * VW:(n + 1) * VW],
                    stot[:, m:m + 1])
                nc.sync.dma_start(
                    o_probs[m * 128:(m + 1) * 128,
                            n * VW:(n + 1) * VW], ob[:])
        ph5p.release()
        ph5w.release()
        ph5.release()
        dram.release()
        work.release()
        const.release()

    n = legalize_waits(nc)
    if os.environ.get("BASS_LSTM_VERBOSE"):
        print(f"[kernel] legalized {n} waits")
    return nc


_CACHE = {}


def _get_program(debug=False):
    key = ("prog", debug)
    if key not in _CACHE:
        _CACHE[key] = build_program(debug=debug)
    return _CACHE[key]


def regate(w):
    """Reorder gate blocks along the last axis: (i, f, g, o) -> (i, f, o, g)."""
    i, f, g, o = np.split(np.asarray(w), 4, axis=-1)
    return np.concatenate([i, f, o, g], axis=-1)


def make_in_maps(input_seq, output_seq, enc_emb, dec_emb,
                 Wx_f, Wh_f, b_f, Wx_b, Wh_b, b_b,
                 Wx_d, Wh_d, b_d, attn_scale, Wo, bo):
    bf = ml_dtypes.bfloat16
    Wx_f, Wh_f, b_f = regate(Wx_f), regate(Wh_f), regate(b_f)
    Wx_b, Wh_b, b_b = regate(Wx_b), regate(Wh_b), regate(b_b)
    Wx_d, Wh_d, b_d = regate(Wx_d), regate(Wh_d), regate(b_d)
    assert not np.any(np.asarray(bo)), "bo != 0 not supported by this build"

    def mini_and_idx(emb, seq):
        ids = np.asarray(seq).reshape(-1)              # (b, t) flat
        uniq, inv = np.unique(ids, return_inverse=True)
        mini = np.zeros((NTOK, E), np.float32)
        mini[:len(uniq)] = np.asarray(emb)[uniq]
        idx_col = inv.astype(np.int32).reshape(EM, 128).T.copy()  # [128, EM]
        return mini, idx_col

    enc_mini_f, enc_idx_f = mini_and_idx(enc_emb, input_seq)
    enc_mini_r, enc_idx_r = mini_and_idx(enc_emb,
                                         np.asarray(input_seq)[:, ::-1])
    dec_mini, dec_idx = mini_and_idx(dec_emb, output_seq)

    def bias_cols(bvec, nm):
        return np.asarray(bvec, np.float32).reshape(nm, 128).T.copy()

    shared = dict(
        dec_mini=dec_mini, dec_idx=dec_idx,
        wx_d=np.asarray(Wx_d).astype(bf), wh_d=np.asarray(Wh_d).astype(bf),
        b_d=bias_cols(b_d, MD),
        v_sc=np.asarray(attn_scale, np.float32).reshape(KD, 128).T
        .astype(bf).copy(),
    )
    fwdw = dict(wx_m=np.asarray(Wx_f).astype(bf),
                wh_m=np.asarray(Wh_f).astype(bf), b_m=bias_cols(b_f, ME))
    bwdw = dict(wx_m=np.asarray(Wx_b).astype(bf),
                wh_m=np.asarray(Wh_b).astype(bf), b_m=bias_cols(b_b, ME))
    Wo_np = np.asarray(Wo)
    in_maps = []
    for c in range(NC):
        m = dict(shared)
        if c < 4:
            m.update(fwdw)
            m.update(enc_mini=enc_mini_f, enc_idx=enc_idx_f)
        else:
            m.update(bwdw)
            m.update(enc_mini=enc_mini_r, enc_idx=enc_idx_r)
        m["wo_sh"] = Wo_np[:, c * VSH:(c + 1) * VSH].astype(bf)
        m["tok"] = (np.arange(TPC, dtype=np.int32)
                    + c * TPC).reshape(TPC, 1)
        in_maps.append(m)
    return in_maps


def assemble_output(results):
    out = np.empty((B, TOUT, V), np.float32)
    # gathered token order: r = c2*64 + b*16 + tl ; t = 16*c2 + tl
    r = np.arange(NTOK)
    c2, rem = r // TOKC, r % TOKC
    bb, tl = rem // TPC, rem % TPC
    tt = c2 * TPC + tl
    for c in range(NC):
        out[bb, tt, c * VSH:(c + 1) * VSH] = results[c]["o_probs"]
    return out


def kernel(**inputs):
    debug = bool(os.environ.get("BASS_LSTM_DEBUG"))
    nc = _get_program(debug=debug)
    in_maps = make_in_maps(**inputs)
    last_exc = None
    for attempt in range(4):
        try:
            res = run_bass_kernel_spmd(nc, in_maps, list(range(NC)))
            break
        except Exception as e:  # transient NRT/axon failures
            last_exc = e
            import time as _t
            _t.sleep(5 * (attempt + 1))
    else:
        raise last_exc
    out = assemble_output(res.results)
    if debug:
        kernel.last_results = res.results
    return out


def build_floor_program():
    """Same I/O signature as the real program, near-zero device work —
    used to subtract host/axon dispatch+staging overhead when timing."""
    nc = bass.Bass("TRN2", target_bir_lowering=False, debug=False,
                   num_devices=NC)

    def din(name, shape, dt=FP32):
        return nc.dram_tensor(name, shape, dt, kind="ExternalInput").ap()

    din("enc_mini", [NTOK, E]); din("enc_idx", [128, EM], I32)
    din("dec_mini", [NTOK, E]); din("dec_idx", [128, EM], I32)
    din("wx_m", [E, 4 * H], BF16); din("wh_m", [H, 4 * H], BF16)
    din("b_m", [128, ME])
    din("wx_d", [E, 4 * D], BF16); din("wh_d", [D, 4 * D], BF16)
    din("b_d", [128, MD])
    din("v_sc", [128, KD], BF16); din("wo_sh", [D, VSH], BF16)
    din("tok", [TPC, 1], I32)
    o_probs = nc.dram_tensor("o_probs", [NTOK, VSH], FP32,
                             kind="ExternalOutput").ap()
    with tile.TileContext(nc) as tc:
        with tc.tile_pool(name="z", bufs=1) as z:
            zt = z.tile([128, VW], FP32)
            nc.vector.memset(zt[:], 0.0)
            nc.sync.dma_start(o_probs[0:128, 0:VW], zt[:])
    legalize_waits(nc)
    return nc


def run_timing(inputs, nruns=10):
    """Return (kernel_min_s, floor_min_s) over nruns dispatches each."""
    import time as _t
    in_maps = make_in_maps(**inputs)
    nc_real = _get_program(debug=False)
    if "floor" not in _CACHE:
        _CACHE["floor"] = build_floor_program()
    nc_floor = _CACHE["floor"]
    cores = list(range(NC))
    run_bass_kernel_spmd(nc_real, in_maps, cores)   # warm both
    run_bass_kernel_spmd(nc_floor, in_maps, cores)
    tk, tf = [], []
    for _ in range(nruns):
        t0 = _t.perf_counter()
        run_bass_kernel_spmd(nc_real, in_maps, cores)
        tk.append(_t.perf_counter() - t0)
        t0 = _t.perf_counter()
        run_bass_kernel_spmd(nc_floor, in_maps, cores)
        tf.append(_t.perf_counter() - t0)
    return tk, tf

